# revision 1
# baseline (speedup 1.0000x reference)
"""BGCF layer forward on 8 Trainium2 NeuronCores (Bass/Tile).

Strategy (v3): the reference's h1_item is dead code and the item-side
outputs (h2_item, obs_item) are only read at the sampled pos/neg indices
(<= 2048 unique per branch).  So instead of full 8192x8192 adjacency
passes + ReduceScatter, each core computes:
  - gathered user rows: its 128 batch users' score/adjacency rows
    (h1_user, h2_user, obs_user pieces), and
  - 256 sampled item columns with the FULL 8192-user contraction local
    to the core (no collectives at all).
Adjacency data ships as fp8e4 (0/1 exact), embeddings as bf16 (ie.T for
the score matmuls as fp8); matmuls keep a bf16/fp8 moving operand (4x
faster than fp32 on PE).  Engines execute their instruction streams
serially (including DMA transfer time), so the ACT queue carries only
the exp/finish chain, while SP and Pool stream the loads in
criticality order.  Final tanh + l2norm + index assembly happen on host
(as in the row-sharded baseline), on [1024 x 192] data.
"""
import numpy as np
import ml_dtypes

import concourse.bacc as bacc
import concourse.tile as tile
import concourse.mybir as mybir
from concourse.bass_utils import run_bass_kernel_spmd

F32 = mybir.dt.float32
BF16 = mybir.dt.bfloat16
FP8 = mybir.dt.float8e4
ACT_F = mybir.ActivationFunctionType

NP_BF16 = ml_dtypes.bfloat16
NP_FP8 = ml_dtypes.float8_e4m3

M = 8            # cores
U = 8192         # users
I = 8192         # items
D = 64
B = 1024         # batch
BSH = B // M     # batch rows per core (128)
SEL = 2048       # padded sampled item columns (pos+neg unique <= 2048)
SELC = SEL // M  # sampled columns per core (256)
NCH = U // 128   # user-row chunks (64)
NT = I // 128    # item tiles (64)
G = 8            # gathered groups (8 item tiles each)
GT = NT // G     # tiles per group (8)
EPS = 1e-6

OUT_NAMES = ("HB", "ITA", "ITO")

_CACHE = {}


def _build():
    nc = bacc.Bacc("TRN2", target_bir_lowering=False, debug=False, num_devices=M)

    # ---- inputs (per core) ----
    UEGT = nc.dram_tensor("UEGT", [D, 2, BSH], FP8, kind="ExternalInput")
    IET = nc.dram_tensor("IET", [D, 2, I // 2], FP8, kind="ExternalInput")
    IEA = nc.dram_tensor("IEA", [128, NCH, D + 1], BF16, kind="ExternalInput")
    UEAHR = nc.dram_tensor("UEAHR", [128, NCH // 2, 2, 2 * D], FP8, kind="ExternalInput")
    DEGS = nc.dram_tensor("DEGS", [128, 6], F32, kind="ExternalInput")
    AUTD = nc.dram_tensor("AUTD", [128, NT, BSH], FP8, kind="ExternalInput")
    OUTD = nc.dram_tensor("OUTD", [128, NT, BSH], FP8, kind="ExternalInput")
    ACTD = nc.dram_tensor("ACTD", [128, NCH // 2, 2, SELC], FP8, kind="ExternalInput")
    OCTD = nc.dram_tensor("OCTD", [128, NCH // 2, 2, SELC], FP8, kind="ExternalInput")
    # [:, 0:128] identity, [0:64, 128+64k : 128+64(k+1)] = W1, W2, WOBS
    WID = nc.dram_tensor("WID", [128, 320], BF16, kind="ExternalInput")

    # ---- outputs ----
    HB = nc.dram_tensor("HB", [BSH, 3, D], F32, kind="ExternalOutput")
    ITA = nc.dram_tensor("ITA", [128, SELC // 128, D], F32, kind="ExternalOutput")
    ITO = nc.dram_tensor("ITO", [128, SELC // 128, D], F32, kind="ExternalOutput")

    with tile.TileContext(nc) as tc:
        with tc.tile_pool(name="pscore", bufs=2, space="PSUM") as pscore, \
             tc.tile_pool(name="psacc", bufs=1, space="PSUM") as psacc, \
             tc.tile_pool(name="pscol", bufs=1, space="PSUM") as pscol, \
             tc.tile_pool(name="psfin", bufs=1, space="PSUM") as psfin, \
             tc.tile_pool(name="per", bufs=1) as per, \
             tc.tile_pool(name="stp", bufs=3) as stp, \
             tc.tile_pool(name="finp", bufs=3) as finp:

            # ---- persistent SBUF tiles ----
            wid_sb = per.tile([128, 320], BF16, tag="wid")
            uegt_sb = per.tile([D, 2, BSH], FP8, tag="uegt")
            iet_sb = per.tile([D, 2, I // 2], FP8, tag="iet")
            iea_sb = per.tile([128, NCH, D + 1], BF16, tag="iea")
            ueahr_sb = per.tile([128, NCH // 2, 2, 2 * D], FP8, tag="ueahr")
            degs_sb = per.tile([128, 6], F32, tag="degs")
            aut_sb = per.tile([128, NT, BSH], FP8, tag="aut")
            out_sb = per.tile([128, NT, BSH], FP8, tag="outd")
            act_sb = per.tile([128, NCH // 2, 2, SELC], FP8, tag="actd")
            oct_sb = per.tile([128, NCH // 2, 2, SELC], FP8, tag="octd")
            hb_sb = per.tile([128, 3, D], F32, tag="hb")
            ita_sb = per.tile([128, SELC // 128, D], F32, tag="ita")
            ito_sb = per.tile([128, SELC // 128, D], F32, tag="ito")

            # ---- DMA issue order = per-queue execution order ----
            # SP queue: early-critical gathered feeds, then late column blobs
            nc.sync.dma_start(iet_sb[:], IET[:, :, :])
            nc.sync.dma_start(iea_sb[:, 0:32, :], IEA[:, 0:32, :])
            nc.sync.dma_start(ueahr_sb[:], UEAHR[:, :, :, :])
            nc.sync.dma_start(iea_sb[:, 32:64, :], IEA[:, 32:64, :])
            nc.sync.dma_start(act_sb[:, 16:32, :, :], ACTD[:, 16:32, :, :])
            nc.sync.dma_start(oct_sb[:, 24:32, :, :], OCTD[:, 24:32, :, :])

            # Pool queue
            nc.gpsimd.dma_start(wid_sb[:], WID[:, :])
            nc.gpsimd.dma_start(degs_sb[:], DEGS[:, :])
            nc.gpsimd.dma_start(aut_sb[:, 0:8, :], AUTD[:, 0:8, :])
            nc.gpsimd.dma_start(aut_sb[:, 8:64, :], AUTD[:, 8:64, :])
            nc.gpsimd.dma_start(act_sb[:, 0:16, :, :], ACTD[:, 0:16, :, :])
            nc.gpsimd.dma_start(oct_sb[:, 0:16, :, :], OCTD[:, 0:16, :, :])
            nc.gpsimd.dma_start(out_sb[:, 0:32, :], OUTD[:, 0:32, :])
            nc.gpsimd.dma_start(oct_sb[:, 16:24, :, :], OCTD[:, 16:24, :, :])

            # ACT queue: tiny load, then compute only (plus one late blob)
            nc.scalar.dma_start(uegt_sb[:], UEGT[:, :, :])

            # packed psum accumulators
            ps_acc = psacc.tile([128, 3 * (D + 1)], F32, tag="acc")
            ps_si = ps_acc[:, 0:D + 1]
            ps_au = ps_acc[:, D + 1:2 * (D + 1)]
            ps_ou = ps_acc[:, 2 * (D + 1):3 * (D + 1)]
            ps_col = pscol.tile([128, 4 * D], F32, tag="col")

            # ---- PE warmup: keep the p-state ramp warm until data lands ----
            warm_sb = per.tile([128, 256], BF16, tag="warm")
            nc.vector.memset(warm_sb[:], 0.0)
            for _ in range(14):
                nc.tensor.matmul(ps_col[:], warm_sb[:, 0:128],
                                 warm_sb[:], start=True, stop=True,
                                 skip_group_check=True)

            # preload the exp/tanh/copy activation table with a dummy op
            # (reads the freshly-memset warm tile, so it runs at t~0.2us)
            dummy_sb = finp.tile([D, 16], BF16, tag="dummy")
            nc.scalar.activation(dummy_sb[:], warm_sb[0:D, 0:16], ACT_F.Exp)

            # ---- gathered-batch pass ----
            # groups of item tiles; smaller first groups start the
            # exp->mul->accum chain earlier
            GRPS = [(0, 4), (4, 4), (8, 8), (16, 8), (24, 8), (32, 8),
                    (40, 8), (48, 8), (56, 4), (60, 4)]
            score_tiles = {}

            def score_mms(g):
                t0, n = GRPS[g]
                ps_s = pscore.tile([128, GT * BSH], F32, tag="s")
                for j in range(n):
                    t = t0 + j
                    pb = 32 * (t // 32)
                    tc_ = t % 32
                    nc.tensor.matmul(ps_s[:, j * BSH:(j + 1) * BSH],
                                     iet_sb[pb:pb + 32, :,
                                            tc_ * 128:(tc_ + 1) * 128],
                                     uegt_sb[pb:pb + 32, :, :],
                                     start=True, stop=True,
                                     perf_mode=mybir.MatmulPerfMode.DoubleRow,
                                     skip_group_check=True)
                score_tiles[g] = ps_s

            def gather_exp(g):
                t0, n = GRPS[g]
                ps_s = score_tiles.pop(g)
                st = stp.tile([128, GT * BSH], BF16, tag="st")
                nc.scalar.activation(st[:, 0:n * BSH], ps_s[:, 0:n * BSH],
                                     ACT_F.Exp)
                stm = stp.tile([128, GT * BSH], BF16, tag="stm")
                nc.vector.tensor_mul(stm[:, 0:n * BSH], st[:, 0:n * BSH],
                                     aut_sb[:, t0:t0 + n, :])
                return stm

            def gather_mms(g, stm):
                t0, n = GRPS[g]
                for j in range(n):
                    t = t0 + j
                    nc.tensor.matmul(ps_si[:], stm[:, j * BSH:(j + 1) * BSH],
                                     iea_sb[:, t, :],
                                     start=(t == 0), stop=(t == NT - 1),
                                     skip_group_check=True)
                    nc.tensor.matmul(ps_au[:], aut_sb[:, t, :],
                                     iea_sb[:, t, :],
                                     start=False, stop=(t == NT - 1),
                                     skip_group_check=True)

            # ---- sampled item-column pass (full local contraction) ----
            col_tiles = {("a", 0): ps_col[:, 0:D],
                         ("a", 1): ps_col[:, D:2 * D],
                         ("o", 0): ps_col[:, 2 * D:3 * D],
                         ("o", 1): ps_col[:, 3 * D:4 * D]}

            col_started = [False]

            def col_pass(mat_sb, key, u0, u1):
                # compensated fp8 DoubleRow: 256-deep contraction per MM,
                # hi + residual passes accumulate into the same psum.  The
                # very first MM start=True marks the shared bank; later
                # ranges zero on first touch.
                for tt in range(SELC // 128):
                    ps_c = col_tiles[(key, tt)]
                    for u in range(u0, u1):
                        last = (u == NCH // 2 - 1)
                        first = not col_started[0]
                        col_started[0] = True
                        nc.tensor.matmul(ps_c[:],
                                         mat_sb[:, u, :,
                                                tt * 128:(tt + 1) * 128],
                                         ueahr_sb[:, u, :, 0:D],
                                         start=first, stop=False,
                                         perf_mode=mybir.MatmulPerfMode.DoubleRow,
                                         skip_group_check=True)
                        nc.tensor.matmul(ps_c[:],
                                         mat_sb[:, u, :,
                                                tt * 128:(tt + 1) * 128],
                                         ueahr_sb[:, u, :, D:2 * D],
                                         start=False, stop=last,
                                         perf_mode=mybir.MatmulPerfMode.DoubleRow,
                                         skip_group_check=True)

            def ou_accums(t0, t1):
                for t in range(t0, t1):
                    nc.tensor.matmul(ps_ou[:], out_sb[:, t, :],
                                     iea_sb[:, t, :],
                                     start=False, stop=(t == NT - 1),
                                     skip_group_check=True)

            # ---- finishing: two row-tiles share one transpose ----
            def finish_pair(x2_ap, wofs, funcs, outs, recs):
                # x2_ap: [128, 128] = two 64-wide column blocks in psum
                xc = finp.tile([128, 128], BF16, tag="xc")
                nc.vector.tensor_copy(xc[:], x2_ap)
                pt = psfin.tile([128, 128], BF16, tag="pt")
                nc.tensor.transpose(pt[:], xc[:], wid_sb[:, 0:128])
                xt = finp.tile([128, 128], BF16, tag="xt")
                nc.vector.tensor_copy(xt[:], pt[:])
                ph = psfin.tile([128, 2, D], F32, tag="ph")
                for k in range(2):
                    nc.tensor.matmul(ph[:, k, :], xt[64 * k:64 * (k + 1), :],
                                     wid_sb[64 * k:64 * (k + 1),
                                            128 + wofs[k]:192 + wofs[k]],
                                     start=True, stop=True,
                                     skip_group_check=True)
                    nc.scalar.activation(outs[k], ph[:, k, :], funcs[k],
                                         scale=recs[k])

            # ================= emission schedule =================
            def keep_warm(n):
                for _ in range(n):
                    nc.tensor.matmul(ps_col[:], warm_sb[:, 0:128],
                                     warm_sb[:], start=True, stop=True,
                                     skip_group_check=True)

            score_mms(0)
            score_mms(1)
            gather_mms(0, gather_exp(0))
            score_mms(2)
            gather_mms(1, gather_exp(1))
            score_mms(3)
            gather_mms(2, gather_exp(2))
            score_mms(4)
            gather_mms(3, gather_exp(3))
            score_mms(5)
            gather_mms(4, gather_exp(4))
            score_mms(6)
            col_pass(act_sb, "a", 0, 8)
            gather_mms(5, gather_exp(5))
            score_mms(7)
            col_pass(act_sb, "a", 8, 16)
            gather_mms(6, gather_exp(6))
            score_mms(8)
            gather_mms(7, gather_exp(7))
            score_mms(9)
            gather_mms(8, gather_exp(8))
            gather_mms(9, gather_exp(9))
            # late OBS-row blob rides the ACT queue once the exps are done
            nc.scalar.dma_start(out_sb[:, 32:48, :], OUTD[:, 32:48, :])
            nc.scalar.dma_start(out_sb[:, 48:64, :], OUTD[:, 48:64, :])
            # batch user finishing (si/au complete): h1+h2 share a transpose
            rec2 = finp.tile([128, 1], F32, tag="rec2")
            nc.vector.tensor_scalar_add(rec2[:], ps_acc[:, D:D + 1], EPS)
            recb = finp.tile([128, 1], F32, tag="recb")
            nc.vector.reciprocal(recb[:], rec2[:])
            finish_pair(
                ps_acc[:, 0:2 * (D + 1)].rearrange(
                    "p (k j) -> p k j", j=D + 1)[:, :, 0:D],
                (0, 64), (ACT_F.Copy, ACT_F.Copy),
                (hb_sb[:, 0, :], hb_sb[:, 1, :]),
                (recb[:], degs_sb[:, 4:5]))
            nc.gpsimd.dma_start(HB[:, 0:2, :], hb_sb[:, 0:2, :])

            col_pass(oct_sb, "o", 0, 16)
            col_pass(act_sb, "a", 16, 32)

            # item-side A finishing
            finish_pair(ps_col[:, 0:2 * D], (64, 64),
                        (ACT_F.Copy, ACT_F.Copy),
                        (ita_sb[:, 0, :], ita_sb[:, 1, :]),
                        (degs_sb[:, 0:1], degs_sb[:, 1:2]))
            nc.sync.dma_start(ITA[:, :, :], ita_sb[:])

            ou_accums(0, 32)
            col_pass(oct_sb, "o", 16, 24)
            ou_accums(32, 48)
            col_pass(oct_sb, "o", 24, 32)
            ou_accums(48, NT)

            finish_pair(ps_col[:, 2 * D:4 * D], (128, 128),
                        (ACT_F.Tanh, ACT_F.Tanh),
                        (ito_sb[:, 0, :], ito_sb[:, 1, :]),
                        (degs_sb[:, 2:3], degs_sb[:, 3:4]))
            nc.sync.dma_start(ITO[:, :, :], ito_sb[:])

            # obs-user finishing (single; packs with nothing)
            xco = finp.tile([128, D], BF16, tag="xco")
            nc.vector.tensor_copy(xco[:], ps_ou[:, 0:D])
            pto = psfin.tile([D, 128], BF16, tag="pt", name="pto")
            nc.tensor.transpose(pto[:], xco[:], wid_sb[:, 0:128])
            xto = finp.tile([D, 128], BF16, tag="xt", name="xto")
            nc.vector.tensor_copy(xto[:], pto[:])
            pho = psfin.tile([128, D], F32, tag="ph", name="pho")
            nc.tensor.matmul(pho[:], xto[:], wid_sb[0:D, 256:320],
                             start=True, stop=True, skip_group_check=True)
            nc.scalar.activation(hb_sb[:, 2, :], pho[:], ACT_F.Tanh,
                                 scale=degs_sb[:, 5:6])
            nc.gpsimd.dma_start(HB[:, 2, :], hb_sb[:, 2, :])

    nc.compile()
    return nc


def _get_nc():
    if "nc" not in _CACHE:
        _CACHE["nc"] = _build()
    return _CACHE["nc"]


def _pmaj(x, inner):
    """[8192, inner] row-major -> [128, 64, inner] partition-major."""
    return np.ascontiguousarray(
        x.reshape(NCH, 128, inner).transpose(1, 0, 2))


def _prep_in_maps(users, pos_items, neg_items, adj_matrix, obs_users,
                  obs_pos_items, obs_neg_items, obs_adj_matrix, user_emb,
                  item_emb, W_1, W_2, W_obs):
    adj = np.asarray(adj_matrix, dtype=np.float32)
    oadj = np.asarray(obs_adj_matrix, dtype=np.float32)
    ue = np.asarray(user_emb, dtype=np.float32)
    ie = np.asarray(item_emb, dtype=np.float32)
    users = np.asarray(users).astype(np.int64)
    obs_users = np.asarray(obs_users).astype(np.int64)

    # sampled item columns (padded to SEL)
    pn = np.concatenate([np.asarray(pos_items), np.asarray(neg_items)])
    sel_a, inv_a = np.unique(pn.astype(np.int64), return_inverse=True)
    selp_a = np.zeros(SEL, np.int64)
    selp_a[:len(sel_a)] = sel_a
    on = np.concatenate([np.asarray(obs_pos_items), np.asarray(obs_neg_items)])
    sel_o, inv_o = np.unique(on.astype(np.int64), return_inverse=True)
    selp_o = np.zeros(SEL, np.int64)
    selp_o[:len(sel_o)] = sel_o

    adj_cols = adj[:, selp_a]          # [8192, 2048]
    oadj_cols = oadj[:, selp_o]

    ones_u = np.ones((U, 1), np.float32)
    iea = _pmaj(np.concatenate([ie, ones_u], axis=1), D + 1).astype(NP_BF16)

    def _pairmaj(x, inner):
        """[8192, inner] -> [128, 32, 2, inner], pairs along rows."""
        return np.ascontiguousarray(
            x.reshape(NCH // 2, 128, 2, inner).transpose(1, 0, 2, 3))

    # x64 scaling keeps the fp8 residual out of the subnormal range; the
    # 1/64 is folded into the host-computed degree reciprocals
    ue_s = ue * 64.0
    uea_hi = ue_s.astype(NP_FP8)
    uea_res = (ue_s - uea_hi.astype(np.float32)).astype(NP_FP8)
    ueahr = np.concatenate(
        [_pairmaj(uea_hi.astype(np.float32), D),
         _pairmaj(uea_res.astype(np.float32), D)], axis=3).astype(NP_FP8)
    dega = 1.0 / (64.0 * (adj_cols.sum(axis=0) + EPS))        # [2048]
    dego = 1.0 / (64.0 * (oadj_cols.sum(axis=0) + EPS))
    iet_t = np.ascontiguousarray(ie.T).astype(NP_FP8)        # [64, 8192]
    # DoubleRow pair-interleave: [k, i, m] = ie.T[2k+i, m], halves stacked
    iet = np.concatenate([iet_t[:, 0:I // 2].reshape(32, 2, I // 2),
                          iet_t[:, I // 2:I].reshape(32, 2, I // 2)],
                         axis=0)                              # [64, 2, 4096]

    wid = np.zeros((128, 320), np.float32)
    wid[:, 0:128] = np.eye(128, dtype=np.float32)
    wid[0:D, 128:192] = np.asarray(W_1, dtype=np.float32)
    wid[0:D, 192:256] = np.asarray(W_2, dtype=np.float32)
    wid[0:D, 256:320] = np.asarray(W_obs, dtype=np.float32)
    wid[D:128, 128:320] = wid[0:D, 128:320]
    wid = wid.astype(NP_BF16)

    in_maps = []
    for c in range(M):
        bs = slice(c * BSH, (c + 1) * BSH)
        ub = users[bs]
        ob = obs_users[bs]
        cs = slice(c * SELC, (c + 1) * SELC)
        in_maps.append({
            "UEGT": np.concatenate(
                [np.ascontiguousarray(ue[ub].T).reshape(32, 2, BSH)] * 2,
                axis=0).astype(NP_FP8),
            "IET": iet,
            "IEA": iea,
            "UEAHR": ueahr,
            "DEGS": np.ascontiguousarray(np.stack(
                [dega[cs][0:128], dega[cs][128:256],
                 dego[cs][0:128], dego[cs][128:256],
                 1.0 / (adj[ub].sum(axis=1) + EPS),
                 1.0 / (oadj[ob].sum(axis=1) + EPS)], axis=1)).astype(np.float32),
            "AUTD": _pmaj(np.ascontiguousarray(adj[ub].T), BSH).astype(NP_FP8),
            "OUTD": _pmaj(np.ascontiguousarray(oadj[ob].T), BSH).astype(NP_FP8),
            "ACTD": _pairmaj(np.ascontiguousarray(adj_cols[:, cs]), SELC).astype(NP_FP8),
            "OCTD": _pairmaj(np.ascontiguousarray(oadj_cols[:, cs]), SELC).astype(NP_FP8),
            "WID": wid,
        })
    return in_maps, inv_a, inv_o


def _assemble(results, inv_a, inv_o):
    hb = np.concatenate([np.asarray(r["HB"]).reshape(BSH, 3, D)
                         for r in results], axis=0)
    h1, h2u, obsu = hb[:, 0, :], hb[:, 1, :], hb[:, 2, :]

    def unpack(x):
        return (np.asarray(x).reshape(128, SELC // 128, D)
                .transpose(1, 0, 2).reshape(SELC, D))

    ita = np.concatenate([unpack(r["ITA"]) for r in results], axis=0)
    ito = np.concatenate([unpack(r["ITO"]) for r in results], axis=0)

    h2_pos = ita[inv_a[:B]]
    h2_neg = ita[inv_a[B:]]
    obs_pos = ito[inv_o[:B]]
    obs_neg = ito[inv_o[B:]]

    def l2n(x):
        n = np.sqrt((x * x).sum(axis=1, keepdims=True))
        return x / np.maximum(n, 1e-12)

    h_user = np.tanh(np.concatenate([h1, h2u, obsu], axis=1))
    h_pos = np.tanh(np.concatenate([h2_pos, h2_pos, obs_pos], axis=1))
    h_neg = np.tanh(np.concatenate([h2_neg, h2_neg, obs_neg], axis=1))
    return l2n(h_user), l2n(h_pos), l2n(h_neg)


def kernel(users, pos_items, neg_items, adj_matrix, obs_users, obs_pos_items,
           obs_neg_items, obs_adj_matrix, iteration, user_emb, item_emb,
           W_1, W_2, W_obs):
    nc = _get_nc()
    in_maps, inv_a, inv_o = _prep_in_maps(
        users, pos_items, neg_items, adj_matrix, obs_users, obs_pos_items,
        obs_neg_items, obs_adj_matrix, user_emb, item_emb, W_1, W_2, W_obs)
    res = run_bass_kernel_spmd(nc, in_maps, core_ids=list(range(M)))
    return _assemble(res.results, inv_a, inv_o)



# revision 38
# speedup vs baseline: 1.0980x; 1.0980x over previous
"""BGCF layer forward on 8 Trainium2 NeuronCores (Bass/Tile).

Strategy (v5): each core computes, for its shard, the raw aggregation
sums (all O(N^2) contractions) on device; the host applies the tiny
O(N*D^2) per-row normalizations/projections (degree scaling, 64x64 W
matmuls, tanh, l2norm, index gathers) during unsharding:
  - score pass: s = ue_b @ ie.T in fp8 DoubleRow (iet relaid out to
    128 partitions), masked softmax numerators/denominator via
    exp + 30*adj trick (groups 0..5 mask on DVE, 6..7 on PE by
    accumulating 30*adj into the score psum and biasing the exp).
  - gather pass: si (coef numerator + denominator via ones column),
    au (adjacency row sums), ou (obs rows) accumulate in one psum bank.
  - item pass: 256 sampled columns per core, full 8192-user
    contraction, fp8 hi+residual DoubleRow.
Cost-model-aware scheduling: three DMA queues (SP/Pool/ACT) carry
~13us each; ACT's DMA rides BEFORE its exp chain (the scheduling pass
would freeze late fillers ahead of the exps); adjacency columns load
early/assorted so PE's mid-kernel holes absorb the column matmuls; all
engines are kept busy across producer boundaries to dodge the
idle-wakeup penalty (+1717ns on DMA edges).
Outputs (raw sums): HB [128,3,65] = si|den, au, ou; ITAO [128,4,64].
"""
import math
import numpy as np
import ml_dtypes

import concourse.bacc as bacc
import concourse.tile as tile
import concourse.mybir as mybir
from concourse.bass_utils import run_bass_kernel_spmd

F32 = mybir.dt.float32
BF16 = mybir.dt.bfloat16
FP8 = mybir.dt.float8e4
ACT_F = mybir.ActivationFunctionType
DR = mybir.MatmulPerfMode.DoubleRow

NP_BF16 = ml_dtypes.bfloat16
NP_FP8 = ml_dtypes.float8_e4m3

M = 8            # cores
U = 8192         # users
I = 8192         # items
D = 64
B = 1024         # batch
BSH = B // M     # batch rows per core (128)
SEL = 2048       # padded sampled item columns
SELC = SEL // M  # sampled columns per core (256)
NCH = U // 128   # user-row chunks (64)
NT = 64          # item tiles
EPS = 1e-6
AMP = 30.0       # adjacency amplitude (exact in fp8e4m3)
EXPB = -AMP + math.log(AMP)   # exp bias for PE-masked groups
NG = 8           # score/exp groups
GT = NT // NG    # tiles per group (8)
PE_MASK_G = (6, 7)   # groups masked via PE bias instead of DVE mul

OUT_NAMES = ("HB", "ITAO")

_CACHE = {}


def _build():
    nc = bacc.Bacc("TRN2", target_bir_lowering=False, debug=False, num_devices=M)

    # ---- inputs (per core) ----
    IET3 = nc.dram_tensor("IET3", [64, 2, 4096], FP8, kind="ExternalInput")
    UEGT3 = nc.dram_tensor("UEGT3", [64, 2, 128], FP8, kind="ExternalInput")
    IEA = nc.dram_tensor("IEA", [128, NT, D + 1], BF16, kind="ExternalInput")
    UEAHR = nc.dram_tensor("UEAHR", [128, NCH // 2, 2, 2 * D], FP8, kind="ExternalInput")
    AUTD = nc.dram_tensor("AUTD", [128, NT, BSH], FP8, kind="ExternalInput")   # 30*adj
    OUTD = nc.dram_tensor("OUTD", [128, NT, BSH], FP8, kind="ExternalInput")
    ACTD = nc.dram_tensor("ACTD", [128, NCH // 2, 2, SELC], FP8, kind="ExternalInput")
    OCTD = nc.dram_tensor("OCTD", [128, NCH // 2, 2, SELC], FP8, kind="ExternalInput")
    IDN = nc.dram_tensor("IDN", [128, 128], BF16, kind="ExternalInput")

    # ---- outputs (raw sums; host normalizes/projects) ----
    HB = nc.dram_tensor("HB", [BSH, 3, D + 1], F32, kind="ExternalOutput")
    ITAO = nc.dram_tensor("ITAO", [128, 4, D], F32, kind="ExternalOutput")

    with tile.TileContext(nc) as tc:
        with tc.tile_pool(name="pscore", bufs=3, space="PSUM") as pscore, \
             tc.tile_pool(name="psacc", bufs=1, space="PSUM") as psacc, \
             tc.tile_pool(name="pscol", bufs=1, space="PSUM") as pscol, \
             tc.tile_pool(name="per", bufs=1) as per, \
             tc.tile_pool(name="stp", bufs=3) as stp:

            # ---- persistent SBUF tiles ----
            iet_sb = per.tile([64, 2, 4096], FP8, tag="iet")
            uegt_sb = per.tile([64, 2, 128], FP8, tag="uegt")
            iea_sb = per.tile([128, NT, D + 1], BF16, tag="iea")
            ueahr_sb = per.tile([128, NCH // 2, 2, 2 * D], FP8, tag="ueahr")
            aut_sb = per.tile([128, NT, BSH], FP8, tag="aut")
            out_sb = per.tile([128, NT, BSH], FP8, tag="outd")
            act_sb = per.tile([128, NCH // 2, 2, SELC], FP8, tag="actd")
            oct_sb = per.tile([128, NCH // 2, 2, SELC], FP8, tag="octd")
            idn_sb = per.tile([128, 128], BF16, tag="idn")
            hb_sb = per.tile([128, 3, D + 1], F32, tag="hb")
            itao_sb = per.tile([128, 4, D], F32, tag="itao")
            warm_sb = per.tile([128, 128], BF16, tag="warm")
            expb_sb = per.tile([128, 1], F32, tag="expb")

            # ---- PSUM accumulators: one pool (bank) per open group ----
            ps_sia = psacc.tile([128, 3, D + 1], F32, tag="sia")   # si|au|ou
            ps_ou = ps_sia[:, 2, 0:D]
            ps_c4 = pscol.tile([128, 4, D], F32, tag="c4")  # item A|O cols
            ps_ca = ps_c4[:, 0:2, :]
            ps_co = ps_c4[:, 2:4, :]

            # ---- SP queue ----
            nc.sync.dma_start(iet_sb[:, :, 0:2048], IET3[:, :, 0:2048])
            nc.sync.dma_start(iet_sb[:, :, 2048:4096], IET3[:, :, 2048:4096])
            nc.sync.dma_start(act_sb[:, 0:8, :, :], ACTD[:, 0:8, :, :])
            nc.sync.dma_start(iea_sb[:, 0:32, :], IEA[:, 0:32, :])
            nc.sync.dma_start(act_sb[:, 8:16, :, :], ACTD[:, 8:16, :, :])
            nc.sync.dma_start(iea_sb[:, 32:64, :], IEA[:, 32:64, :])
            nc.sync.dma_start(act_sb[:, 16:32, :, :], ACTD[:, 16:32, :, :])
            nc.sync.dma_start(oct_sb[:, 20:26, :, :], OCTD[:, 20:26, :, :])

            # ---- Pool queue ----
            nc.gpsimd.dma_start(uegt_sb[:], UEGT3[:, :, :])
            nc.gpsimd.dma_start(idn_sb[:], IDN[:, :])
            nc.gpsimd.dma_start(aut_sb[:, 0:16, :], AUTD[:, 0:16, :])
            nc.gpsimd.dma_start(ueahr_sb[:, 0:16, :, :], UEAHR[:, 0:16, :, :])
            nc.gpsimd.dma_start(oct_sb[:, 0:8, :, :], OCTD[:, 0:8, :, :])
            nc.gpsimd.dma_start(aut_sb[:, 16:32, :], AUTD[:, 16:32, :])
            nc.gpsimd.dma_start(oct_sb[:, 8:16, :, :], OCTD[:, 8:16, :, :])
            nc.gpsimd.dma_start(aut_sb[:, 32:64, :], AUTD[:, 32:64, :])
            nc.gpsimd.dma_start(out_sb[:, 0:32, :], OUTD[:, 0:32, :])
            nc.gpsimd.dma_start(ueahr_sb[:, 16:32, :, :], UEAHR[:, 16:32, :, :])
            nc.gpsimd.dma_start(oct_sb[:, 16:20, :, :], OCTD[:, 16:20, :, :])

            # ---- ACT queue: early loads before the exp chain ----
            nc.scalar.dma_start(out_sb[:, 32:64, :], OUTD[:, 32:64, :])
            nc.scalar.dma_start(oct_sb[:, 26:32, :, :], OCTD[:, 26:32, :, :])

            # ---- engine bridges + bank opening ----
            nc.vector.memset(warm_sb[:], 0.0)
            nc.vector.memset(expb_sb[:], EXPB)
            dvebridge_sb = per.tile([128, 3700], BF16, tag="dveb")
            nc.vector.memset(dvebridge_sb[:], 0.0)
            ps_warm = ps_co[0:64, 0, :]
            for _ in range(13):
                nc.tensor.matmul(ps_warm, warm_sb[:, 0:64], warm_sb[:, 0:64],
                                 start=True, stop=True, skip_group_check=True)
            # open accumulator banks: zero-touch every region once (first
            # mm of each bank start=True); later accumulations start=False
            for k in range(3):
                nc.tensor.matmul(ps_sia[:, k, :], warm_sb[:],
                                 warm_sb[:, 0:D + 1], start=(k == 0),
                                 stop=False, skip_group_check=True)
            for k in range(4):
                nc.tensor.matmul(ps_c4[:, k, :], warm_sb[:],
                                 warm_sb[:, 0:D], start=(k == 0), stop=False,
                                 skip_group_check=True)

            # ---- score pass ----
            score_tiles = {}

            def score_mms(g):
                pe_mask = g in PE_MASK_G
                ps_s = pscore.tile([128, GT * BSH], F32, tag="s")
                for j in range(GT):
                    t = g * GT + j
                    b, sl = (0, t) if t < 32 else (1, t - 32)
                    nc.tensor.matmul(ps_s[:, j * BSH:(j + 1) * BSH],
                                     iet_sb[32 * b:32 * b + 32, :,
                                            sl * 128:(sl + 1) * 128],
                                     uegt_sb[32 * b:32 * b + 32, :, :],
                                     start=True, stop=not pe_mask,
                                     perf_mode=DR, skip_group_check=True)
                    if pe_mask:
                        nc.tensor.matmul(ps_s[:, j * BSH:(j + 1) * BSH],
                                         idn_sb[:], aut_sb[:, t, :],
                                         start=False, stop=True,
                                         skip_group_check=True)
                score_tiles[g] = ps_s

            def exp_g(g):
                """For PE-masked groups the exp output IS stm (30*exp*adj)."""
                ps_s = score_tiles.pop(g)
                st = stp.tile([128, GT, BSH], BF16, tag="st")
                if g in PE_MASK_G:
                    nc.scalar.activation(st[:].rearrange("p a b -> p (a b)"),
                                         ps_s[:], ACT_F.Exp, bias=expb_sb[:])
                else:
                    nc.scalar.activation(st[:].rearrange("p a b -> p (a b)"),
                                         ps_s[:], ACT_F.Exp)
                return st

            def mul_g(g, st):
                stm = stp.tile([128, GT, BSH], BF16, tag="stm")
                nc.vector.tensor_mul(stm[:], st[:],
                                     aut_sb[:, g * GT:(g + 1) * GT, :])
                return stm

            def gather_mms(g, stm):
                for j in range(GT):
                    t = g * GT + j
                    nc.tensor.matmul(ps_sia[:, 0, :], stm[:, j, :],
                                     iea_sb[:, t, :],
                                     start=False, stop=(t == NT - 1),
                                     skip_group_check=True)
                    nc.tensor.matmul(ps_sia[:, 1, 0:D], aut_sb[:, t, :],
                                     iea_sb[:, t, 0:D],
                                     start=False, stop=(t == NT - 1),
                                     skip_group_check=True)

            def ou_mms(t0, t1):
                for t in range(t0, t1):
                    nc.tensor.matmul(ps_ou, out_sb[:, t, :],
                                     iea_sb[:, t, 0:D],
                                     start=False, stop=(t == NT - 1),
                                     skip_group_check=True)

            def col_pass(mat_sb, ps_c, u0, u1):
                for tt in range(SELC // 128):
                    for u in range(u0, u1):
                        last = (u == NCH // 2 - 1)
                        nc.tensor.matmul(ps_c[:, tt, :],
                                         mat_sb[:, u, :,
                                                tt * 128:(tt + 1) * 128],
                                         ueahr_sb[:, u, :, 0:D],
                                         start=False, stop=False,
                                         perf_mode=DR, skip_group_check=True)
                        nc.tensor.matmul(ps_c[:, tt, :],
                                         mat_sb[:, u, :,
                                                tt * 128:(tt + 1) * 128],
                                         ueahr_sb[:, u, :, D:2 * D],
                                         start=False, stop=last,
                                         perf_mode=DR, skip_group_check=True)

            # ================= emission schedule =================
            sts = {}
            stms = {}

            def stage(g):
                sts[g] = exp_g(g)
                if g in PE_MASK_G:
                    stms[g] = sts[g]
                else:
                    stms[g] = mul_g(g, sts[g])

            score_mms(0)
            score_mms(1)
            stage(0)
            score_mms(2)
            stage(1)
            score_mms(3)
            gather_mms(0, stms[0])
            stage(2)
            score_mms(4)
            gather_mms(1, stms[1])
            col_pass(oct_sb, ps_co, 0, 4)
            stage(3)
            score_mms(5)
            gather_mms(2, stms[2])
            col_pass(act_sb, ps_ca, 0, 8)
            stage(4)
            score_mms(6)
            gather_mms(3, stms[3])
            col_pass(oct_sb, ps_co, 4, 12)
            stage(5)
            score_mms(7)
            gather_mms(4, stms[4])
            col_pass(act_sb, ps_ca, 8, 16)
            ou_mms(0, 32)
            stage(6)
            gather_mms(5, stms[5])
            col_pass(oct_sb, ps_co, 12, 20)
            ou_mms(32, NT)
            stage(7)
            gather_mms(6, stms[6])
            col_pass(act_sb, ps_ca, 16, 24)
            gather_mms(7, stms[7])
            col_pass(oct_sb, ps_co, 20, 26)
            col_pass(act_sb, ps_ca, 24, 32)

            # user/obs raw sums out (split: ou region can complete early)
            nc.vector.tensor_copy(hb_sb[:, 2, :], ps_sia[:, 2, :])
            nc.vector.tensor_copy(hb_sb[:, 0:2, :], ps_sia[:, 0:2, :])
            nc.sync.dma_start(HB[:, :, :], hb_sb[:])

            # item-A raw sums out
            nc.vector.tensor_copy(itao_sb[:, 0:2, :], ps_ca[:])

            col_pass(oct_sb, ps_co, 26, 32)
            nc.vector.tensor_copy(itao_sb[:, 2:4, :], ps_co[:])
            nc.scalar.dma_start(ITAO[:, :, :], itao_sb[:])

    nc.compile()
    return nc


def _get_nc():
    if "nc" not in _CACHE:
        _CACHE["nc"] = _build()
    return _CACHE["nc"]


def _pmaj(x, inner):
    """[8192, inner] row-major -> [128, 64, inner] partition-major."""
    return np.ascontiguousarray(
        x.reshape(NCH, 128, inner).transpose(1, 0, 2))


def _pairmaj(x, inner):
    """[8192, inner] -> [128, 32, 2, inner], pairs along rows."""
    return np.ascontiguousarray(
        x.reshape(NCH // 2, 128, 2, inner).transpose(1, 0, 2, 3))


def _prep_in_maps(users, pos_items, neg_items, adj_matrix, obs_users,
                  obs_pos_items, obs_neg_items, obs_adj_matrix, user_emb,
                  item_emb, W_1, W_2, W_obs):
    adj = np.asarray(adj_matrix, dtype=np.float32)
    oadj = np.asarray(obs_adj_matrix, dtype=np.float32)
    ue = np.asarray(user_emb, dtype=np.float32)
    ie = np.asarray(item_emb, dtype=np.float32)
    users = np.asarray(users).astype(np.int64)
    obs_users = np.asarray(obs_users).astype(np.int64)

    # sampled item columns (padded to SEL)
    pn = np.concatenate([np.asarray(pos_items), np.asarray(neg_items)])
    sel_a, inv_a = np.unique(pn.astype(np.int64), return_inverse=True)
    selp_a = np.zeros(SEL, np.int64)
    selp_a[:len(sel_a)] = sel_a
    on = np.concatenate([np.asarray(obs_pos_items), np.asarray(obs_neg_items)])
    sel_o, inv_o = np.unique(on.astype(np.int64), return_inverse=True)
    selp_o = np.zeros(SEL, np.int64)
    selp_o[:len(sel_o)] = sel_o

    adj_cols = adj[:, selp_a]          # [8192, 2048]
    oadj_cols = oadj[:, selp_o]

    ones_u = np.ones((U, 1), np.float32)
    iea = _pmaj(np.concatenate([ie, ones_u], axis=1), D + 1).astype(NP_BF16)

    # x64 scaling keeps the fp8 residual out of the subnormal range
    ue_s = ue * 64.0
    uea_hi = ue_s.astype(NP_FP8)
    uea_res = (ue_s - uea_hi.astype(np.float32)).astype(NP_FP8)
    ueahr = np.concatenate(
        [_pairmaj(uea_hi.astype(np.float32), D),
         _pairmaj(uea_res.astype(np.float32), D)], axis=3).astype(NP_FP8)

    # IET3: ie.T fp8, DR pairs (d = 2k+i); 3 partition blocks at legal
    # bases {0,32,64} holding tiles [0:22), [22:43), [43:64)
    iet_t = np.ascontiguousarray(ie.T).astype(NP_FP8)        # [64, 8192]
    ietr = iet_t.reshape(32, 2, 64, 128)                     # [k, i, t, m]
    iet3 = np.ascontiguousarray(
        ietr.reshape(32, 2, 2, 32, 128).transpose(2, 0, 1, 3, 4)
    ).reshape(64, 2, 4096)

    idn = np.eye(128, dtype=np.float32).astype(NP_BF16)

    in_maps = []
    meta = []
    for c in range(M):
        bs = slice(c * BSH, (c + 1) * BSH)
        ub = users[bs]
        ob = obs_users[bs]
        cs = slice(c * SELC, (c + 1) * SELC)
        uegt = np.ascontiguousarray(ue[ub].T).astype(NP_FP8).reshape(32, 2, BSH)
        in_maps.append({
            "IET3": iet3,
            "UEGT3": np.ascontiguousarray(
                np.broadcast_to(uegt[None], (2, 32, 2, BSH))).reshape(64, 2, BSH),
            "IEA": iea,
            "UEAHR": ueahr,
            "AUTD": _pmaj(np.ascontiguousarray(adj[ub].T) * AMP, BSH).astype(NP_FP8),
            "OUTD": _pmaj(np.ascontiguousarray(oadj[ob].T), BSH).astype(NP_FP8),
            "ACTD": _pairmaj(np.ascontiguousarray(adj_cols[:, cs]), SELC).astype(NP_FP8),
            "OCTD": _pairmaj(np.ascontiguousarray(oadj_cols[:, cs]), SELC).astype(NP_FP8),
            "IDN": idn,
        })
        meta.append({
            "deg_u": adj[ub].sum(axis=1),
            "odeg_u": oadj[ob].sum(axis=1),
        })
    dega = 1.0 / (64.0 * (adj_cols.sum(axis=0) + EPS))        # [2048]
    dego = 1.0 / (64.0 * (oadj_cols.sum(axis=0) + EPS))
    return in_maps, (inv_a, inv_o, meta, dega, dego)


def _assemble(results, aux, W_1, W_2, W_obs):
    inv_a, inv_o, meta, dega, dego = aux
    W_1 = np.asarray(W_1, np.float32)
    W_2 = np.asarray(W_2, np.float32)
    W_obs = np.asarray(W_obs, np.float32)

    h1_l, h2u_l, obsu_l = [], [], []
    for c, r in enumerate(results):
        hb = np.asarray(r["HB"]).reshape(BSH, 3, D + 1)
        num, den = hb[:, 0, 0:D], hb[:, 0, D]
        au, ou = hb[:, 1, 0:D], hb[:, 2, 0:D]
        h1_l.append((num / (den + AMP * EPS)[:, None]) @ W_1)
        h2u_l.append((au / (AMP * (meta[c]["deg_u"] + EPS))[:, None]) @ W_2)
        obsu_l.append((ou / (meta[c]["odeg_u"] + EPS)[:, None]) @ W_obs)
    h1 = np.concatenate(h1_l, axis=0)
    h2u = np.concatenate(h2u_l, axis=0)
    obsu = np.tanh(np.concatenate(obsu_l, axis=0))

    def unpack(x, k0):
        x = np.asarray(x).reshape(128, 4, D)
        return x[:, k0:k0 + 2, :].transpose(1, 0, 2).reshape(SELC, D)

    ita_raw = np.concatenate([unpack(r["ITAO"], 0) for r in results], axis=0)
    ito_raw = np.concatenate([unpack(r["ITAO"], 2) for r in results], axis=0)
    ita = (ita_raw * dega[:, None]) @ W_2
    ito = np.tanh((ito_raw * dego[:, None]) @ W_obs)

    h2_pos = ita[inv_a[:B]]
    h2_neg = ita[inv_a[B:]]
    obs_pos = ito[inv_o[:B]]
    obs_neg = ito[inv_o[B:]]

    def l2n(x):
        n = np.sqrt((x * x).sum(axis=1, keepdims=True))
        return x / np.maximum(n, 1e-12)

    h_user = np.tanh(np.concatenate([h1, h2u, obsu], axis=1))
    h_pos = np.tanh(np.concatenate([h2_pos, h2_pos, obs_pos], axis=1))
    h_neg = np.tanh(np.concatenate([h2_neg, h2_neg, obs_neg], axis=1))
    return l2n(h_user), l2n(h_pos), l2n(h_neg)


def kernel(users, pos_items, neg_items, adj_matrix, obs_users, obs_pos_items,
           obs_neg_items, obs_adj_matrix, iteration, user_emb, item_emb,
           W_1, W_2, W_obs):
    nc = _get_nc()
    in_maps, aux = _prep_in_maps(
        users, pos_items, neg_items, adj_matrix, obs_users, obs_pos_items,
        obs_neg_items, obs_adj_matrix, user_emb, item_emb, W_1, W_2, W_obs)
    res = run_bass_kernel_spmd(nc, in_maps, core_ids=list(range(M)))
    return _assemble(res.results, aux, W_1, W_2, W_obs)


# revision 40
# speedup vs baseline: 1.0997x; 1.0016x over previous
"""BGCF layer forward on 8 Trainium2 NeuronCores (Bass/Tile).

Strategy (v5): each core computes, for its shard, the raw aggregation
sums (all O(N^2) contractions) on device; the host applies the tiny
O(N*D^2) per-row normalizations/projections (degree scaling, 64x64 W
matmuls, tanh, l2norm, index gathers) during unsharding:
  - score pass: s = ue_b @ ie.T in fp8 DoubleRow (iet relaid out to
    128 partitions), masked softmax numerators/denominator via
    exp + 30*adj trick (groups 0..5 mask on DVE, 6..7 on PE by
    accumulating 30*adj into the score psum and biasing the exp).
  - gather pass: si (coef numerator + denominator via ones column),
    au (adjacency row sums), ou (obs rows) accumulate in one psum bank.
  - item pass: 256 sampled columns per core, full 8192-user
    contraction, fp8 hi+residual DoubleRow.
Cost-model-aware scheduling: three DMA queues (SP/Pool/ACT) carry
~13us each; ACT's DMA rides BEFORE its exp chain (the scheduling pass
would freeze late fillers ahead of the exps); adjacency columns load
early/assorted so PE's mid-kernel holes absorb the column matmuls; all
engines are kept busy across producer boundaries to dodge the
idle-wakeup penalty (+1717ns on DMA edges).
Outputs (raw sums): HB [128,3,65] = si|den, au, ou; ITAO [128,4,64].
"""
import math
import numpy as np
import ml_dtypes

import concourse.bacc as bacc
import concourse.tile as tile
import concourse.mybir as mybir
from concourse.bass_utils import run_bass_kernel_spmd

F32 = mybir.dt.float32
BF16 = mybir.dt.bfloat16
FP8 = mybir.dt.float8e4
ACT_F = mybir.ActivationFunctionType
DR = mybir.MatmulPerfMode.DoubleRow

NP_BF16 = ml_dtypes.bfloat16
NP_FP8 = ml_dtypes.float8_e4m3

M = 8            # cores
U = 8192         # users
I = 8192         # items
D = 64
B = 1024         # batch
BSH = B // M     # batch rows per core (128)
SEL = 2048       # padded sampled item columns
SELC = SEL // M  # sampled columns per core (256)
NCH = U // 128   # user-row chunks (64)
NT = 64          # item tiles
EPS = 1e-6
AMP = 30.0       # adjacency amplitude (exact in fp8e4m3)
EXPB = -AMP + math.log(AMP)   # exp bias for PE-masked groups
NG = 8           # score/exp groups
GT = NT // NG    # tiles per group (8)
PE_MASK_G = (6, 7)   # groups masked via PE bias instead of DVE mul

OUT_NAMES = ("HB", "ITAO")

_CACHE = {}


def _build():
    nc = bacc.Bacc("TRN2", target_bir_lowering=False, debug=False, num_devices=M)

    # ---- inputs (per core) ----
    IET3 = nc.dram_tensor("IET3", [64, 2, 4096], FP8, kind="ExternalInput")
    UEGT3 = nc.dram_tensor("UEGT3", [64, 2, 128], FP8, kind="ExternalInput")
    IEA = nc.dram_tensor("IEA", [128, NT, D + 1], BF16, kind="ExternalInput")
    UEAHR = nc.dram_tensor("UEAHR", [128, NCH // 2, 2, 2 * D], FP8, kind="ExternalInput")
    AUTD = nc.dram_tensor("AUTD", [128, NT, BSH], FP8, kind="ExternalInput")   # 30*adj
    OUTD = nc.dram_tensor("OUTD", [128, NT, BSH], FP8, kind="ExternalInput")
    ACTD = nc.dram_tensor("ACTD", [128, NCH // 2, 2, SELC], FP8, kind="ExternalInput")
    OCTD = nc.dram_tensor("OCTD", [128, NCH // 2, 2, SELC], FP8, kind="ExternalInput")
    IDN = nc.dram_tensor("IDN", [128, 128], BF16, kind="ExternalInput")

    # ---- outputs (raw sums; host normalizes/projects) ----
    HB = nc.dram_tensor("HB", [BSH, 3, D + 1], F32, kind="ExternalOutput")
    ITAO = nc.dram_tensor("ITAO", [128, 4, D], F32, kind="ExternalOutput")

    with tile.TileContext(nc) as tc:
        with tc.tile_pool(name="pscore", bufs=3, space="PSUM") as pscore, \
             tc.tile_pool(name="psacc", bufs=1, space="PSUM") as psacc, \
             tc.tile_pool(name="pscol", bufs=1, space="PSUM") as pscol, \
             tc.tile_pool(name="per", bufs=1) as per, \
             tc.tile_pool(name="stp", bufs=3) as stp:

            # ---- persistent SBUF tiles ----
            iet_sb = per.tile([64, 2, 4096], FP8, tag="iet")
            uegt_sb = per.tile([64, 2, 128], FP8, tag="uegt")
            iea_sb = per.tile([128, NT, D + 1], BF16, tag="iea")
            ueahr_sb = per.tile([128, NCH // 2, 2, 2 * D], FP8, tag="ueahr")
            aut_sb = per.tile([128, NT, BSH], FP8, tag="aut")
            out_sb = per.tile([128, NT, BSH], FP8, tag="outd")
            act_sb = per.tile([128, NCH // 2, 2, SELC], FP8, tag="actd")
            oct_sb = per.tile([128, NCH // 2, 2, SELC], FP8, tag="octd")
            idn_sb = per.tile([128, 128], BF16, tag="idn")
            hb_sb = per.tile([128, 3, D + 1], F32, tag="hb")
            itao_sb = per.tile([128, 4, D], F32, tag="itao")
            warm_sb = per.tile([128, 128], BF16, tag="warm")
            expb_sb = per.tile([128, 1], F32, tag="expb")

            # ---- PSUM accumulators: one pool (bank) per open group ----
            ps_sia = psacc.tile([128, 3, D + 1], F32, tag="sia")   # si|au|ou
            ps_ou = ps_sia[:, 2, 0:D]
            ps_c4 = pscol.tile([128, 4, D], F32, tag="c4")  # item A|O cols
            ps_ca = ps_c4[:, 0:2, :]
            ps_co = ps_c4[:, 2:4, :]

            # ---- SP queue ----
            nc.sync.dma_start(iet_sb[:, :, 0:2048], IET3[:, :, 0:2048])
            nc.sync.dma_start(iet_sb[:, :, 2048:4096], IET3[:, :, 2048:4096])
            nc.sync.dma_start(act_sb[:, 0:8, :, :], ACTD[:, 0:8, :, :])
            nc.sync.dma_start(iea_sb[:, 0:32, :], IEA[:, 0:32, :])
            nc.sync.dma_start(act_sb[:, 8:16, :, :], ACTD[:, 8:16, :, :])
            nc.sync.dma_start(iea_sb[:, 32:64, :], IEA[:, 32:64, :])
            nc.sync.dma_start(act_sb[:, 16:32, :, :], ACTD[:, 16:32, :, :])
            nc.sync.dma_start(oct_sb[:, 20:26, :, :], OCTD[:, 20:26, :, :])

            # ---- Pool queue ----
            nc.gpsimd.dma_start(uegt_sb[:], UEGT3[:, :, :])
            nc.gpsimd.dma_start(idn_sb[:], IDN[:, :])
            nc.gpsimd.dma_start(aut_sb[:, 0:16, :], AUTD[:, 0:16, :])
            nc.gpsimd.dma_start(ueahr_sb[:, 0:16, :, :], UEAHR[:, 0:16, :, :])
            nc.gpsimd.dma_start(oct_sb[:, 0:8, :, :], OCTD[:, 0:8, :, :])
            nc.gpsimd.dma_start(aut_sb[:, 16:32, :], AUTD[:, 16:32, :])
            nc.gpsimd.dma_start(oct_sb[:, 8:16, :, :], OCTD[:, 8:16, :, :])
            nc.gpsimd.dma_start(aut_sb[:, 32:64, :], AUTD[:, 32:64, :])
            nc.gpsimd.dma_start(out_sb[:, 0:32, :], OUTD[:, 0:32, :])
            nc.gpsimd.dma_start(ueahr_sb[:, 16:32, :, :], UEAHR[:, 16:32, :, :])
            nc.gpsimd.dma_start(oct_sb[:, 16:20, :, :], OCTD[:, 16:20, :, :])

            # ---- ACT queue: early loads before the exp chain ----
            nc.scalar.dma_start(out_sb[:, 32:64, :], OUTD[:, 32:64, :])
            nc.scalar.dma_start(oct_sb[:, 26:32, :, :], OCTD[:, 26:32, :, :])

            # ---- engine bridges + bank opening ----
            nc.vector.memset(warm_sb[:], 0.0)
            nc.vector.memset(expb_sb[:], EXPB)
            dvebridge_sb = per.tile([128, 3700], BF16, tag="dveb")
            nc.vector.memset(dvebridge_sb[:], 0.0)
            ps_warm = ps_co[0:64, 0, :]
            for _ in range(13):
                nc.tensor.matmul(ps_warm, warm_sb[:, 0:64], warm_sb[:, 0:64],
                                 start=True, stop=True, skip_group_check=True)
            # open accumulator banks: zero-touch every region once (first
            # mm of each bank start=True); later accumulations start=False
            for k in range(3):
                nc.tensor.matmul(ps_sia[:, k, :], warm_sb[:],
                                 warm_sb[:, 0:D + 1], start=(k == 0),
                                 stop=False, skip_group_check=True)
            for k in range(4):
                nc.tensor.matmul(ps_c4[:, k, :], warm_sb[:],
                                 warm_sb[:, 0:D], start=(k == 0), stop=False,
                                 skip_group_check=True)

            # ---- score pass ----
            score_tiles = {}

            def score_mms(g):
                pe_mask = g in PE_MASK_G
                ps_s = pscore.tile([128, GT * BSH], F32, tag="s")
                for j in range(GT):
                    t = g * GT + j
                    b, sl = (0, t) if t < 32 else (1, t - 32)
                    nc.tensor.matmul(ps_s[:, j * BSH:(j + 1) * BSH],
                                     iet_sb[32 * b:32 * b + 32, :,
                                            sl * 128:(sl + 1) * 128],
                                     uegt_sb[32 * b:32 * b + 32, :, :],
                                     start=True, stop=not pe_mask,
                                     perf_mode=DR, skip_group_check=True)
                    if pe_mask:
                        nc.tensor.matmul(ps_s[:, j * BSH:(j + 1) * BSH],
                                         idn_sb[:], aut_sb[:, t, :],
                                         start=False, stop=True,
                                         skip_group_check=True)
                score_tiles[g] = ps_s

            def exp_g(g):
                """For PE-masked groups the exp output IS stm (30*exp*adj)."""
                ps_s = score_tiles.pop(g)
                st = stp.tile([128, GT, BSH], BF16, tag="st")
                if g in PE_MASK_G:
                    nc.scalar.activation(st[:].rearrange("p a b -> p (a b)"),
                                         ps_s[:], ACT_F.Exp, bias=expb_sb[:])
                else:
                    nc.scalar.activation(st[:].rearrange("p a b -> p (a b)"),
                                         ps_s[:], ACT_F.Exp)
                return st

            def mul_g(g, st):
                stm = stp.tile([128, GT, BSH], BF16, tag="stm")
                nc.vector.tensor_mul(stm[:], st[:],
                                     aut_sb[:, g * GT:(g + 1) * GT, :])
                return stm

            def gather_mms(g, stm):
                for j in range(GT):
                    t = g * GT + j
                    nc.tensor.matmul(ps_sia[:, 0, :], stm[:, j, :],
                                     iea_sb[:, t, :],
                                     start=False, stop=(t == NT - 1),
                                     skip_group_check=True)
                    nc.tensor.matmul(ps_sia[:, 1, 0:D], aut_sb[:, t, :],
                                     iea_sb[:, t, 0:D],
                                     start=False, stop=(t == NT - 1),
                                     skip_group_check=True)

            def ou_mms(t0, t1):
                for t in range(t0, t1):
                    nc.tensor.matmul(ps_ou, out_sb[:, t, :],
                                     iea_sb[:, t, 0:D],
                                     start=False, stop=(t == NT - 1),
                                     skip_group_check=True)

            def col_pass(mat_sb, ps_c, u0, u1):
                for tt in range(SELC // 128):
                    for u in range(u0, u1):
                        last = (u == NCH // 2 - 1)
                        nc.tensor.matmul(ps_c[:, tt, :],
                                         mat_sb[:, u, :,
                                                tt * 128:(tt + 1) * 128],
                                         ueahr_sb[:, u, :, 0:D],
                                         start=False, stop=False,
                                         perf_mode=DR, skip_group_check=True)
                        nc.tensor.matmul(ps_c[:, tt, :],
                                         mat_sb[:, u, :,
                                                tt * 128:(tt + 1) * 128],
                                         ueahr_sb[:, u, :, D:2 * D],
                                         start=False, stop=last,
                                         perf_mode=DR, skip_group_check=True)

            # ================= emission schedule =================
            sts = {}
            stms = {}

            def stage(g):
                sts[g] = exp_g(g)
                if g in PE_MASK_G:
                    stms[g] = sts[g]
                else:
                    stms[g] = mul_g(g, sts[g])

            score_mms(0)
            score_mms(1)
            stage(0)
            score_mms(2)
            stage(1)
            score_mms(3)
            gather_mms(0, stms[0])
            stage(2)
            score_mms(4)
            gather_mms(1, stms[1])
            col_pass(oct_sb, ps_co, 0, 4)
            stage(3)
            score_mms(5)
            gather_mms(2, stms[2])
            col_pass(act_sb, ps_ca, 0, 8)
            stage(4)
            score_mms(6)
            gather_mms(3, stms[3])
            col_pass(oct_sb, ps_co, 4, 12)
            stage(5)
            score_mms(7)
            gather_mms(4, stms[4])
            col_pass(act_sb, ps_ca, 8, 16)
            ou_mms(0, 32)
            stage(6)
            gather_mms(5, stms[5])
            col_pass(oct_sb, ps_co, 12, 20)
            ou_mms(32, NT)
            stage(7)
            gather_mms(6, stms[6])
            col_pass(act_sb, ps_ca, 16, 24)
            gather_mms(7, stms[7])
            col_pass(oct_sb, ps_co, 20, 26)
            col_pass(act_sb, ps_ca, 24, 32)

            # user/obs raw sums out (split: ou region can complete early)
            nc.scalar.activation(hb_sb[:, 2, :], ps_sia[:, 2, :], ACT_F.Copy)
            nc.scalar.activation(hb_sb[:, 0:2, :], ps_sia[:, 0:2, :], ACT_F.Copy)
            nc.sync.dma_start(HB[:, :, :], hb_sb[:])

            # item-A raw sums out
            nc.scalar.activation(itao_sb[:, 0:2, :], ps_ca[:], ACT_F.Copy)

            col_pass(oct_sb, ps_co, 26, 32)
            nc.scalar.activation(itao_sb[:, 2:4, :], ps_co[:], ACT_F.Copy)
            nc.scalar.dma_start(ITAO[:, :, :], itao_sb[:])

    nc.compile()
    return nc


def _get_nc():
    if "nc" not in _CACHE:
        _CACHE["nc"] = _build()
    return _CACHE["nc"]


def _pmaj(x, inner):
    """[8192, inner] row-major -> [128, 64, inner] partition-major."""
    return np.ascontiguousarray(
        x.reshape(NCH, 128, inner).transpose(1, 0, 2))


def _pairmaj(x, inner):
    """[8192, inner] -> [128, 32, 2, inner], pairs along rows."""
    return np.ascontiguousarray(
        x.reshape(NCH // 2, 128, 2, inner).transpose(1, 0, 2, 3))


def _prep_in_maps(users, pos_items, neg_items, adj_matrix, obs_users,
                  obs_pos_items, obs_neg_items, obs_adj_matrix, user_emb,
                  item_emb, W_1, W_2, W_obs):
    adj = np.asarray(adj_matrix, dtype=np.float32)
    oadj = np.asarray(obs_adj_matrix, dtype=np.float32)
    ue = np.asarray(user_emb, dtype=np.float32)
    ie = np.asarray(item_emb, dtype=np.float32)
    users = np.asarray(users).astype(np.int64)
    obs_users = np.asarray(obs_users).astype(np.int64)

    # sampled item columns (padded to SEL)
    pn = np.concatenate([np.asarray(pos_items), np.asarray(neg_items)])
    sel_a, inv_a = np.unique(pn.astype(np.int64), return_inverse=True)
    selp_a = np.zeros(SEL, np.int64)
    selp_a[:len(sel_a)] = sel_a
    on = np.concatenate([np.asarray(obs_pos_items), np.asarray(obs_neg_items)])
    sel_o, inv_o = np.unique(on.astype(np.int64), return_inverse=True)
    selp_o = np.zeros(SEL, np.int64)
    selp_o[:len(sel_o)] = sel_o

    adj_cols = adj[:, selp_a]          # [8192, 2048]
    oadj_cols = oadj[:, selp_o]

    ones_u = np.ones((U, 1), np.float32)
    iea = _pmaj(np.concatenate([ie, ones_u], axis=1), D + 1).astype(NP_BF16)

    # x64 scaling keeps the fp8 residual out of the subnormal range
    ue_s = ue * 64.0
    uea_hi = ue_s.astype(NP_FP8)
    uea_res = (ue_s - uea_hi.astype(np.float32)).astype(NP_FP8)
    ueahr = np.concatenate(
        [_pairmaj(uea_hi.astype(np.float32), D),
         _pairmaj(uea_res.astype(np.float32), D)], axis=3).astype(NP_FP8)

    # IET3: ie.T fp8, DR pairs (d = 2k+i); 3 partition blocks at legal
    # bases {0,32,64} holding tiles [0:22), [22:43), [43:64)
    iet_t = np.ascontiguousarray(ie.T).astype(NP_FP8)        # [64, 8192]
    ietr = iet_t.reshape(32, 2, 64, 128)                     # [k, i, t, m]
    iet3 = np.ascontiguousarray(
        ietr.reshape(32, 2, 2, 32, 128).transpose(2, 0, 1, 3, 4)
    ).reshape(64, 2, 4096)

    idn = np.eye(128, dtype=np.float32).astype(NP_BF16)

    in_maps = []
    meta = []
    for c in range(M):
        bs = slice(c * BSH, (c + 1) * BSH)
        ub = users[bs]
        ob = obs_users[bs]
        cs = slice(c * SELC, (c + 1) * SELC)
        uegt = np.ascontiguousarray(ue[ub].T).astype(NP_FP8).reshape(32, 2, BSH)
        in_maps.append({
            "IET3": iet3,
            "UEGT3": np.ascontiguousarray(
                np.broadcast_to(uegt[None], (2, 32, 2, BSH))).reshape(64, 2, BSH),
            "IEA": iea,
            "UEAHR": ueahr,
            "AUTD": _pmaj(np.ascontiguousarray(adj[ub].T) * AMP, BSH).astype(NP_FP8),
            "OUTD": _pmaj(np.ascontiguousarray(oadj[ob].T), BSH).astype(NP_FP8),
            "ACTD": _pairmaj(np.ascontiguousarray(adj_cols[:, cs]), SELC).astype(NP_FP8),
            "OCTD": _pairmaj(np.ascontiguousarray(oadj_cols[:, cs]), SELC).astype(NP_FP8),
            "IDN": idn,
        })
        meta.append({
            "deg_u": adj[ub].sum(axis=1),
            "odeg_u": oadj[ob].sum(axis=1),
        })
    dega = 1.0 / (64.0 * (adj_cols.sum(axis=0) + EPS))        # [2048]
    dego = 1.0 / (64.0 * (oadj_cols.sum(axis=0) + EPS))
    return in_maps, (inv_a, inv_o, meta, dega, dego)


def _assemble(results, aux, W_1, W_2, W_obs):
    inv_a, inv_o, meta, dega, dego = aux
    W_1 = np.asarray(W_1, np.float32)
    W_2 = np.asarray(W_2, np.float32)
    W_obs = np.asarray(W_obs, np.float32)

    h1_l, h2u_l, obsu_l = [], [], []
    for c, r in enumerate(results):
        hb = np.asarray(r["HB"]).reshape(BSH, 3, D + 1)
        num, den = hb[:, 0, 0:D], hb[:, 0, D]
        au, ou = hb[:, 1, 0:D], hb[:, 2, 0:D]
        h1_l.append((num / (den + AMP * EPS)[:, None]) @ W_1)
        h2u_l.append((au / (AMP * (meta[c]["deg_u"] + EPS))[:, None]) @ W_2)
        obsu_l.append((ou / (meta[c]["odeg_u"] + EPS)[:, None]) @ W_obs)
    h1 = np.concatenate(h1_l, axis=0)
    h2u = np.concatenate(h2u_l, axis=0)
    obsu = np.tanh(np.concatenate(obsu_l, axis=0))

    def unpack(x, k0):
        x = np.asarray(x).reshape(128, 4, D)
        return x[:, k0:k0 + 2, :].transpose(1, 0, 2).reshape(SELC, D)

    ita_raw = np.concatenate([unpack(r["ITAO"], 0) for r in results], axis=0)
    ito_raw = np.concatenate([unpack(r["ITAO"], 2) for r in results], axis=0)
    ita = (ita_raw * dega[:, None]) @ W_2
    ito = np.tanh((ito_raw * dego[:, None]) @ W_obs)

    h2_pos = ita[inv_a[:B]]
    h2_neg = ita[inv_a[B:]]
    obs_pos = ito[inv_o[:B]]
    obs_neg = ito[inv_o[B:]]

    def l2n(x):
        n = np.sqrt((x * x).sum(axis=1, keepdims=True))
        return x / np.maximum(n, 1e-12)

    h_user = np.tanh(np.concatenate([h1, h2u, obsu], axis=1))
    h_pos = np.tanh(np.concatenate([h2_pos, h2_pos, obs_pos], axis=1))
    h_neg = np.tanh(np.concatenate([h2_neg, h2_neg, obs_neg], axis=1))
    return l2n(h_user), l2n(h_pos), l2n(h_neg)


def kernel(users, pos_items, neg_items, adj_matrix, obs_users, obs_pos_items,
           obs_neg_items, obs_adj_matrix, iteration, user_emb, item_emb,
           W_1, W_2, W_obs):
    nc = _get_nc()
    in_maps, aux = _prep_in_maps(
        users, pos_items, neg_items, adj_matrix, obs_users, obs_pos_items,
        obs_neg_items, obs_adj_matrix, user_emb, item_emb, W_1, W_2, W_obs)
    res = run_bass_kernel_spmd(nc, in_maps, core_ids=list(range(M)))
    return _assemble(res.results, aux, W_1, W_2, W_obs)
